# revision 30
# baseline (speedup 1.0000x reference)
"""Trainium2 Bass kernel for nn_BLTModel (BLT encoder-decoder), 8 NeuronCores.

Core c -> batch b=c//2, token-half h=c%2. Patch-embed gathers + encoder +
decoder are data-parallel over batch (4 pairs) with a token split inside each
pair; per-pair bf16 AllGathers exchange activations. The final projection is
vocab-sharded: one 8-rank AllGather of the decoder output, then each core
computes logits [4096, 4000] bf16 for its vocab slice and the host
concatenates. Activations are kept transposed (xT [D, tok]) so every linear
is a direct PE matmul; LayerNorm runs in natural layout with the row-sum
fused into the residual add (scalar_tensor_tensor accum) and the rest on the
Scalar engine (Square/Rsqrt/scaled-copy). Attention softmax: exp with key
mask folded as a log-bias, causal mask as 0/1 multiply, denominators via a
ones column appended to V, normalized after the head loop with one batched
ACT reciprocal + gpsimd partition broadcasts. Engine balance: PE matmuls,
ACT exp/LN, DVE evac/adds, GpSimd bias adds + broadcasts.
"""
import contextlib
import sys

if '/opt/trn_rl_repo' not in sys.path:
    sys.path.insert(0, '/opt/trn_rl_repo')

import numpy as np
import ml_dtypes

import concourse.bass as bass
import concourse.mybir as mybir
import concourse.tile as tile
from concourse import bacc
from concourse.bass_utils import run_bass_kernel_spmd
from concourse.masks import make_identity

FP32 = mybir.dt.float32
BF16 = mybir.dt.bfloat16
I32 = mybir.dt.int32
BF = ml_dtypes.bfloat16
AF = mybir.ActivationFunctionType
OP = mybir.AluOpType
AX = mybir.AxisListType

B, P, T, K = 4, 256, 1024, 8
D, FFD, NH = 512, 2048, 8
LE, LD = 2, 2
VOCAB, BUCKETS = 32000, 50000
HD = D // NH
N_CORES = 8
PP = P // 2
TOK = T // 2
NK = D // 128
NKF = FFD // 128
VS = VOCAB // N_CORES
VT = 500
NVT = VS // VT
NEG = -1e9
EPS = 1e-5
PAIRS = [[2 * i, 2 * i + 1] for i in range(4)]
ALL8 = [list(range(N_CORES))]

_CACHE = {}


def build_program(debug=False):
    nc = bacc.Bacc("TRN2", target_bir_lowering=False, debug=False,
                   num_devices=N_CORES)

    # ---- inputs ----
    t_tables = nc.dram_tensor("tables_st", [3 * BUCKETS, D], BF16, kind="ExternalInput")
    t_ngids = nc.dram_tensor("ng_ids", [PP, 3 * K], I32, kind="ExternalInput")
    t_ppos = nc.dram_tensor("ppos_own", [PP, D], FP32, kind="ExternalInput")
    t_pmask = nc.dram_tensor("pmask_own", [PP, 1], FP32, kind="ExternalInput")
    t_pmln = nc.dram_tensor("pmask_ln", [128, 2], FP32, kind="ExternalInput")
    t_temb = nc.dram_tensor("token_emb", [VOCAB, D], BF16, kind="ExternalInput")
    t_tall = nc.dram_tensor("tids_all", [128, 8], I32, kind="ExternalInput")
    t_town = nc.dram_tensor("tids_own", [128, 4], I32, kind="ExternalInput")
    t_tposT = nc.dram_tensor("tposT", [D, T], BF16, kind="ExternalInput")
    t_tpos_own = nc.dram_tensor("tpos_own", [TOK, D], FP32, kind="ExternalInput")
    t_tmln = nc.dram_tensor("tmask_ln", [128, 8], FP32, kind="ExternalInput")
    t_causal = nc.dram_tensor("causal01", [T, TOK], BF16, kind="ExternalInput")

    t_encW = nc.dram_tensor("enc_Wp", [LE, D, 4 * D], BF16, kind="ExternalInput")
    t_encW1 = nc.dram_tensor("enc_W1", [LE, D, FFD], BF16, kind="ExternalInput")
    t_encW2 = nc.dram_tensor("enc_W2", [LE, FFD, D], BF16, kind="ExternalInput")
    t_saW = nc.dram_tensor("dec_saWp", [LD, D, 4 * D], BF16, kind="ExternalInput")
    t_caW = nc.dram_tensor("dec_caWp", [LD, D, 4 * D], BF16, kind="ExternalInput")
    t_decW1 = nc.dram_tensor("dec_W1", [LD, D, FFD], BF16, kind="ExternalInput")
    t_decW2 = nc.dram_tensor("dec_W2", [LD, FFD, D], BF16, kind="ExternalInput")
    t_wout = nc.dram_tensor("wout", [D, VS], BF16, kind="ExternalInput")

    t_qkvb = nc.dram_tensor("qkvb", [6, 3, NK, 128], FP32, kind="ExternalInput")
    t_ob = nc.dram_tensor("ob_bc", [6, 128, D], FP32, kind="ExternalInput")
    t_b1 = nc.dram_tensor("b1pp", [4, NKF, 128], FP32, kind="ExternalInput")
    t_b2 = nc.dram_tensor("b2_bc", [4, 128, D], FP32, kind="ExternalInput")
    t_lng = nc.dram_tensor("ln_g_bc", [10, 128, D], FP32, kind="ExternalInput")
    t_lnb = nc.dram_tensor("ln_b_bc", [10, 128, D], FP32, kind="ExternalInput")
    t_lngT = nc.dram_tensor("ln_gT", [10, 128, NK], FP32, kind="ExternalInput")
    t_lnbT = nc.dram_tensor("ln_bT", [10, 128, NK], FP32, kind="ExternalInput")
    t_boutb = nc.dram_tensor("bout_bc", [128, VS], FP32, kind="ExternalInput")

    t_out = nc.dram_tensor("logits", [NVT, B * T // 128, 128, VT], BF16, kind="ExternalOutput")
    dbg = {}
    if debug:
        dbg['pe'] = nc.dram_tensor("d_pe", [PP, D], FP32, kind="ExternalOutput")
        dbg['enc0'] = nc.dram_tensor("d_enc0", [PP, D], FP32, kind="ExternalOutput")
        dbg['mem'] = nc.dram_tensor("d_mem", [PP, D], FP32, kind="ExternalOutput")
        dbg['y0'] = nc.dram_tensor("d_y0", [TOK, D], FP32, kind="ExternalOutput")
        dbg['dec0'] = nc.dram_tensor("d_dec0", [TOK, D], FP32, kind="ExternalOutput")
        dbg['dec1'] = nc.dram_tensor("d_dec1", [TOK, D], FP32, kind="ExternalOutput")

    ps_stack = contextlib.ExitStack()

    with tile.TileContext(nc) as tc, \
         tc.tile_pool(name="const", bufs=1) as const, \
         tc.tile_pool(name="sml", bufs=2) as sml, \
         tc.tile_pool(name="mid", bufs=1) as mid, \
         tc.tile_pool(name="dram", bufs=1, space="DRAM") as dram:

        ps_lin = ps_stack.enter_context(tc.tile_pool(name="ps_lin", bufs=2, space="PSUM"))
        ps_sc = ps_stack.enter_context(tc.tile_pool(name="ps_sc", bufs=2, space="PSUM"))
        ps_z = ps_stack.enter_context(tc.tile_pool(name="ps_z", bufs=2, space="PSUM"))
        ps_tp = ps_stack.enter_context(tc.tile_pool(name="ps_tp", bufs=2, space="PSUM"))

        # ================= constants (small, whole-program) =================
        ident = const.tile([128, 128], FP32)
        make_identity(nc, ident[:])
        identb = const.tile([128, 128], BF16)
        make_identity(nc, identb[:])
        epsc = const.tile([128, 1], FP32)
        nc.vector.memset(epsc[:], EPS)
        pmln = const.tile([128, 2], FP32)
        nc.sync.dma_start(pmln[:], t_pmln[:])
        tmln = const.tile([128, 8], FP32)
        nc.sync.dma_start(tmln[:], t_tmln[:])
        qkvb = const.tile([128, 6, 3, NK], FP32)
        nc.sync.dma_start(qkvb[:], t_qkvb[:].rearrange("a m g p -> p a m g"))
        b1c = const.tile([128, 4, NKF], FP32)
        nc.sync.dma_start(b1c[:], t_b1[:].rearrange("a g p -> p a g"))
        lngT = const.tile([128, 10, NK], FP32)
        nc.sync.dma_start(lngT[:], t_lngT[:].rearrange("i p k -> p i k"))
        lnbT = const.tile([128, 10, NK], FP32)
        nc.sync.dma_start(lnbT[:], t_lnbT[:].rearrange("i p k -> p i k"))
        ones1r = const.tile([1, 128], FP32)
        nc.vector.memset(ones1r[:], 1.0)

        # ============ persistent activations (cross-phase) ============
        pe_nat = mid.tile([128, 1, D], FP32, tag="enc_nat_a")
        enc_nat_b = mid.tile([128, 1, D], FP32, tag="enc_nat_b")
        dec_x = mid.tile([128, 4, D], FP32, tag="dec_nat_a")
        dec_nat_b = mid.tile([128, 4, D], FP32, tag="dec_nat_b")
        dec_nat_c = mid.tile([128, 4, D], FP32, tag="dec_nat_c")
        y0T = mid.tile([128, NK, T], BF16, tag="y0T")
        memT_holder = []

        # ================= helpers =================
        def transpose_into(xT_dst, src_nat, tg0, ngrp, src_bf=False):
            idt = identb if src_bf else ident
            lp = (nc.allow_low_precision(reason="bf16 transpose, no accumulation")
                  if src_bf else contextlib.nullcontext())
            with lp:
                i = 0
                for g in range(ngrp):
                    for kk in range(NK):
                        tp = ps_tp.tile([128, 128], BF16 if src_bf else FP32, tag="tp")
                        nc.tensor.transpose(
                            tp[:], src_nat[:, g, kk * 128:(kk + 1) * 128], idt[:])
                        dst = xT_dst[:, kk, (tg0 + g) * 128:(tg0 + g + 1) * 128]
                        if i % 2 == 0:
                            nc.scalar.copy(dst, tp[:])
                        else:
                            nc.vector.tensor_copy(out=dst, in_=tp[:])
                        i += 1

        def layer_norm(pool, sum_nat, ngrp, ln_i, s1s, xT_dst, natural=True):
            """Fused LN + transpose. Writes normalized, affine-transformed,
            transposed output to xT_dst (bf16). The per-token 1/sigma scaling
            rides the transpose matmul as a diag(rs) rhs; the per-feature g/b
            affine rides the PSUM evacuation. If `natural`, also materializes
            the LN output in sum_nat (needed when it feeds a residual)."""
            if natural:
                lg_t = pool.tile([128, D], FP32, tag="lng")
                nc.sync.dma_start(lg_t[:], t_lng[ln_i])
                lb_t = pool.tile([128, D], FP32, tag="lnb")
                nc.sync.dma_start(lb_t[:], t_lnb[ln_i])
            for g in range(ngrp):
                x = sum_nat[:, g, :]
                negmu = sml.tile([128, 1], FP32, tag="ln_nm", bufs=4)
                nc.scalar.mul(negmu[:], s1s[g][:], -1.0 / D)
                sq = sml.tile([128, D], FP32, tag="ln_sq", bufs=1)
                ss = sml.tile([128, 1], FP32, tag="ln_ss", bufs=4)
                nc.scalar.activation(sq[:], x, AF.Square, bias=negmu[:, :1],
                                     accum_out=ss[:, :1])
                sd = sml.tile([128, 1], FP32, tag="ln_sd", bufs=4)
                nc.scalar.activation(sd[:], ss[:], AF.Sqrt, bias=epsc[:, :1],
                                     scale=1.0 / D)
                rs = sml.tile([128, 1], FP32, tag="ln_rs", bufs=4)
                nc.vector.reciprocal(rs[:], sd[:])
                xc = sml.tile([128, D], FP32, tag="ln_xc", bufs=3)
                nc.vector.tensor_scalar(out=xc[:], in0=x, scalar1=negmu[:, :1],
                                        scalar2=None, op0=OP.add)
                diag = sml.tile([128, 128], FP32, tag="ln_diag", bufs=2)
                nc.vector.tensor_scalar(out=diag[:], in0=ident[:],
                                        scalar1=rs[:, :1], scalar2=None,
                                        op0=OP.mult)
                for kk in range(NK):
                    tp = ps_tp.tile([128, 128], FP32, tag="tp")
                    nc.tensor.matmul(tp[:], xc[:, kk * 128:(kk + 1) * 128],
                                     diag[:], start=True, stop=True)
                    nc.vector.tensor_scalar(
                        out=xT_dst[:, kk, g * 128:(g + 1) * 128], in0=tp[:],
                        scalar1=lngT[:, ln_i, kk:kk + 1],
                        scalar2=lnbT[:, ln_i, kk:kk + 1],
                        op0=OP.mult, op1=OP.add)
                if natural:
                    nc.vector.tensor_scalar(out=x, in0=xc[:], scalar1=rs[:, :1],
                                            scalar2=None, op0=OP.mult)
                    nc.vector.tensor_tensor(out=x, in0=x, in1=lg_t[:], op=OP.mult)
                    nc.vector.tensor_tensor(out=x, in0=x, in1=lb_t[:], op=OP.add)

        def attention(pool, a_i, xT_q, nq, xT_kv, nkv, w_sb, wo_sb, mask_ln,
                      causal_sb, resid_nat, out_sum, s1_tag):
            nkc = nkv // 128
            ntg = nq // 128
            qT = pool.tile([128, NK, nq], BF16, tag="qT")
            kT = pool.tile([128, NK, nkv], BF16, tag="kT")
            for m, dstT, src, ncols in ((0, qT, xT_q, nq), (1, kT, xT_kv, nkv)):
                for g in range(NK):
                    for c0 in range(0, ncols, 512):
                        cw = min(512, ncols - c0)
                        pp = ps_lin.tile([128, 512], FP32, tag="lin")
                        for kk in range(NK):
                            nc.tensor.matmul(
                                pp[:, :cw],
                                w_sb[:, kk, m * D + g * 128: m * D + (g + 1) * 128],
                                src[:, kk, c0:c0 + cw],
                                start=(kk == 0), stop=(kk == NK - 1))
                        nc.vector.tensor_scalar(
                            out=dstT[:, g, c0:c0 + cw], in0=pp[:, :cw],
                            scalar1=qkvb[:, a_i, m, g:g + 1], scalar2=None,
                            op0=OP.add)
            v_sb = pool.tile([128, nkc, NH, HD + 1], BF16, tag="v")
            nc.vector.memset(v_sb[:, :, :, HD:HD + 1], 1.0)
            for c in range(nkc):
                pp = ps_lin.tile([128, 512], FP32, tag="lin")
                for kk in range(NK):
                    nc.tensor.matmul(
                        pp[:], xT_kv[:, kk, c * 128:(c + 1) * 128],
                        w_sb[:, kk, 2 * D:3 * D],
                        start=(kk == 0), stop=(kk == NK - 1))
                nc.vector.tensor_copy(
                    out=v_sb[:, c, :, :HD],
                    in_=pp[:].rearrange("p (h d) -> p h d", h=NH))
            ob_t = pool.tile([128, D], FP32, tag="ob")
            nc.sync.dma_start(ob_t[:], t_ob[a_i])
            zT = pool.tile([64, NH, nq], BF16, tag="zT")
            for hp in range(NH // 2):
                zps = []
                for s in range(2):
                    zp = ps_z.tile([128, 512], FP32, tag="z",
                                   name=f"zp_{a_i}_{hp}_{s}")
                    zps.append(zp)
                for c in range(nkc):
                    sps = []
                    for s in range(2):
                        h = 2 * hp + s
                        pl = (h % 2) * 64
                        gq = h // 2
                        sp = ps_sc.tile([128, 512], FP32, tag="sc",
                                        name=f"sp_{a_i}_{hp}_{c}_{s}")
                        nc.tensor.matmul(
                            sp[:, :nq],
                            kT[pl:pl + 64, gq, c * 128:(c + 1) * 128],
                            qT[pl:pl + 64, gq, :nq],
                            start=True, stop=True)
                        sps.append(sp)
                    for s in range(2):
                        h = 2 * hp + s
                        a_sb = sml.tile([128, 512], BF16, tag="a", bufs=4)
                        nc.scalar.activation(a_sb[:, :nq], sps[s][:, :nq], AF.Exp,
                                             bias=mask_ln[:, c:c + 1],
                                             scale=float(1.0 / np.sqrt(HD)))
                        if causal_sb is not None:
                            nc.vector.tensor_tensor(
                                out=a_sb[:, :nq], in0=a_sb[:, :nq],
                                in1=causal_sb[:, c, :nq], op=OP.mult)
                        nc.tensor.matmul(
                            zps[s][:HD + 1, :nq], v_sb[:, c, h, :], a_sb[:, :nq],
                            start=(c == 0), stop=(c == nkc - 1))
                for s in range(2):
                    h = 2 * hp + s
                    rcin = sml.tile([1, 512], FP32, tag="rcin", bufs=1)
                    nc.scalar.copy(rcin[:, :nq], zps[s][HD:HD + 1, :nq])
                    rc = sml.tile([1, 512], FP32, tag="rc", bufs=2)
                    nc.vector.reciprocal_approx_fast(
                        out=rc[:, :nq], in_=rcin[:, :nq])
                    bcd = pool.tile([64, 512], FP32, tag="bcd", bufs=2)
                    nc.gpsimd.partition_broadcast(bcd[:, :nq], rc[:, :nq])
                    with nc.allow_low_precision(reason="softmax denom recip"):
                        nc.vector.tensor_tensor(out=zT[:, h, :nq],
                                                in0=zps[s][:HD, :nq],
                                                in1=bcd[:, :nq], op=OP.mult)
            s1s = []
            for g in range(ntg):
                op_ = ps_lin.tile([128, 512], FP32, tag="lin")
                for h in range(NH):
                    nc.tensor.matmul(op_[:], zT[:, h, g * 128:(g + 1) * 128],
                                     wo_sb[:, h, :],
                                     start=(h == 0), stop=False)
                nc.tensor.matmul(op_[:], ones1r[:, :], ob_t[0:1, :],
                                 start=False, stop=True)
                s1 = sml.tile([128, 1], FP32, tag=s1_tag, bufs=6)
                nc.vector.scalar_tensor_tensor(
                    out=out_sum[:, g, :], in0=op_[:], scalar=0.0,
                    in1=resid_nat[:, g, :], op0=OP.bypass, op1=OP.add,
                    accum_out=s1[:, :1])
                s1s.append(s1)
            return s1s

        def ffn(pool, ff_i, xT, ntok, w1_sb, w2_sb, resid_nat, out_sum):
            b2_t = pool.tile([128, D], FP32, tag="b2")
            nc.sync.dma_start(b2_t[:], t_b2[ff_i])
            th = min(ntok, 256)
            s1s = []
            for t0 in range(0, ntok, th):
                hT = pool.tile([128, NKF, th], BF16, tag="hT")
                for fg in range(NKF):
                    pp = ps_lin.tile([128, 512], FP32, tag="lin")
                    for kk in range(NK):
                        nc.tensor.matmul(pp[:, :th],
                                         w1_sb[:, kk, fg * 128:(fg + 1) * 128],
                                         xT[:, kk, t0:t0 + th],
                                         start=(kk == 0), stop=(kk == NK - 1))
                    nc.scalar.activation(hT[:, fg, :], pp[:, :th], AF.Relu,
                                         bias=b1c[:, ff_i, fg:fg + 1], scale=1.0)
                for g in range(th // 128):
                    gg = t0 // 128 + g
                    pp = ps_lin.tile([128, 512], FP32, tag="lin")
                    for fg in range(NKF):
                        nc.tensor.matmul(pp[:], hT[:, fg, g * 128:(g + 1) * 128],
                                         w2_sb[:, fg, :],
                                         start=(fg == 0), stop=False)
                    nc.tensor.matmul(pp[:], ones1r[:, :], b2_t[0:1, :],
                                     start=False, stop=True)
                    s1 = sml.tile([128, 1], FP32, tag="s1f", bufs=6)
                    nc.vector.scalar_tensor_tensor(
                        out=out_sum[:, gg, :], in0=pp[:], scalar=0.0,
                        in1=resid_nat[:, gg, :], op0=OP.bypass, op1=OP.add,
                        accum_out=s1[:, :1])
                    s1s.append(s1)
            return s1s

        def pair_ag(dstpool, src_sb, ncols, tag, dst_tag=None):
            bi = dram.tile([NK * 128, ncols], BF16, tag=tag + "_i")
            nc.sync.dma_start(
                bi.opt().rearrange("(k p) t -> p k t", p=128), src_sb[:])
            bo = dram.tile([2 * NK * 128, ncols], BF16, tag=tag + "_o")
            nc.gpsimd.collective_compute(
                "AllGather", OP.bypass, replica_groups=PAIRS,
                ins=[bi.opt()], outs=[bo.opt()])
            dst = dstpool.tile([128, NK, 2 * ncols], BF16,
                               tag=(dst_tag or (tag + "_d")))
            for r in range(2):
                nc.sync.dma_start(
                    dst[:, :, r * ncols:(r + 1) * ncols],
                    bo.opt()[r * NK * 128:(r + 1) * NK * 128]
                    .rearrange("(k p) t -> p k t", p=128))
            return dst

        # ================= stage A: patch embedding =================
        xT_eown = mid.tile([128, NK, PP], BF16, tag="xT_eown")
        with nc.named_scope("A_patch"), tc.tile_pool(name="pa", bufs=1) as pa:
            ngid = pa.tile([PP, 3 * K], I32)
            nc.sync.dma_start(ngid[:], t_ngids[:])
            gth = pa.tile([128, 3 * K, D], BF16, tag="gth")
            for k in range(3 * K):
                nc.gpsimd.indirect_dma_start(
                    out=gth[:, k, :], out_offset=None, in_=t_tables[:],
                    in_offset=bass.IndirectOffsetOnAxis(ap=ngid[:, k:k + 1], axis=0))
            acc0 = pa.tile([128, D], FP32, tag="pacc0")
            acc1 = pa.tile([128, D], FP32, tag="pacc1")
            nc.vector.tensor_tensor(out=acc0[:], in0=gth[:, 0, :], in1=gth[:, 2, :], op=OP.add)
            nc.vector.tensor_tensor(out=acc1[:], in0=gth[:, 1, :], in1=gth[:, 3, :], op=OP.add)
            for k in range(4, 3 * K, 2):
                nc.vector.tensor_tensor(out=acc0[:], in0=acc0[:], in1=gth[:, k, :], op=OP.add)
            for k in range(5, 3 * K, 2):
                nc.vector.tensor_tensor(out=acc1[:], in0=acc1[:], in1=gth[:, k, :], op=OP.add)
            nc.vector.tensor_tensor(out=pe_nat[:, 0, :], in0=acc0[:], in1=acc1[:], op=OP.add)
            pmask = pa.tile([PP, 1], FP32)
            nc.sync.dma_start(pmask[:], t_pmask[:])
            nc.vector.tensor_scalar(out=pe_nat[:, 0, :], in0=pe_nat[:, 0, :],
                                    scalar1=pmask[:, :1], scalar2=None, op0=OP.mult)
            ppos = pa.tile([PP, D], FP32, tag="ppos")
            nc.sync.dma_start(ppos[:], t_ppos[:])
            nc.vector.tensor_tensor(out=pe_nat[:, 0, :], in0=pe_nat[:, 0, :],
                                    in1=ppos[:], op=OP.add)
            if debug:
                nc.sync.dma_start(dbg['pe'][:], pe_nat[:, 0, :])
            transpose_into(xT_eown, pe_nat, 0, 1)

        # ================= stage B: encoder (+ token embedding overlap) ======
        with nc.named_scope("B_enc"), tc.tile_pool(name="pe_", bufs=1) as pw:
            xT_kv_enc = pair_ag(pw, xT_eown, PP, "ag0")

            # ---- token embedding (independent of encoder; fills AG hole) ----
            with nc.named_scope("C_tok"), tc.tile_pool(name="pc", bufs=1) as pc:
                tall = pc.tile([128, 8], I32)
                nc.sync.dma_start(tall[:], t_tall[:])
                town = pc.tile([128, 4], I32)
                nc.sync.dma_start(town[:], t_town[:])
                y0n = pc.tile([128, 8, D], BF16, tag="y0n")
                for c in range(8):
                    nc.gpsimd.indirect_dma_start(
                        out=y0n[:, c, :], out_offset=None, in_=t_temb[:],
                        in_offset=bass.IndirectOffsetOnAxis(ap=tall[:, c:c + 1], axis=0))
                transpose_into(y0T, y0n, 0, 8, src_bf=True)
                tposT = pc.tile([128, NK, T], BF16, tag="tposT")
                nc.sync.dma_start(tposT[:], t_tposT[:].rearrange("(k p) t -> p k t", p=128))
                nc.vector.tensor_tensor(out=y0T[:].rearrange("p k t -> p (k t)"),
                                        in0=y0T[:].rearrange("p k t -> p (k t)"),
                                        in1=tposT[:].rearrange("p k t -> p (k t)"), op=OP.add)
                y0o = pc.tile([128, 4, D], BF16, tag="y0o")
                for c in range(4):
                    nc.gpsimd.indirect_dma_start(
                        out=y0o[:, c, :], out_offset=None, in_=t_temb[:],
                        in_offset=bass.IndirectOffsetOnAxis(ap=town[:, c:c + 1], axis=0))
                tpos_o = pc.tile([128, 4, D], FP32, tag="tpos_o")
                nc.sync.dma_start(tpos_o[:], t_tpos_own[:].rearrange("(g p) n -> p g n", p=128))
                for g in range(4):
                    nc.vector.tensor_tensor(out=dec_x[:, g, :], in0=tpos_o[:, g, :],
                                            in1=y0o[:, g, :], op=OP.add)
                if debug:
                    nc.sync.dma_start(dbg['y0'][:].rearrange("(g p) n -> p g n", p=128),
                                      dec_x[:])

            for l in range(LE):
                w_sb = pw.tile([128, NK, 4 * D], BF16, tag="wqkv")
                nc.sync.dma_start(w_sb[:], t_encW[l].rearrange("(k p) n -> p k n", p=128))
                wo_sb = pw.tile([64, NH, D], BF16, tag="wo")
                nc.sync.dma_start(
                    wo_sb[:], t_encW[l, :, 3 * D:4 * D].rearrange("(h p) n -> p h n", p=64))
                w1_sb = pw.tile([128, NK, FFD], BF16, tag="w1")
                nc.sync.dma_start(w1_sb[:], t_encW1[l].rearrange("(k p) n -> p k n", p=128))
                w2_sb = pw.tile([128, NKF, D], BF16, tag="w2")
                nc.sync.dma_start(w2_sb[:], t_encW2[l].rearrange("(k p) n -> p k n", p=128))

                s1s = attention(pw, l, xT_eown, PP, xT_kv_enc, P, w_sb, wo_sb, pmln,
                                None, pe_nat, enc_nat_b, "s1a")
                xT_mid_t = pw.tile([128, NK, PP], BF16, tag="xT_emid")
                layer_norm(pw, enc_nat_b, 1, 2 * l, s1s, xT_mid_t)
                s1s = ffn(pw, l, xT_mid_t, PP, w1_sb, w2_sb, enc_nat_b, pe_nat)
                xT_eown = mid.tile([128, NK, PP], BF16, tag=f"xT_eo{l}")
                layer_norm(pw, pe_nat, 1, 2 * l + 1, s1s, xT_eown,
                           natural=(l == 0 or debug))
                if l == 0:
                    xT_kv_enc = pair_ag(pw, xT_eown, PP, "ag1")
                    if debug:
                        nc.sync.dma_start(dbg['enc0'][:], pe_nat[:, 0, :])
            memT = pair_ag(mid, xT_eown, PP, "ag2")
            memT_holder.append(memT)
            if debug:
                nc.sync.dma_start(dbg['mem'][:], pe_nat[:, 0, :])

        # ================= stage D: decoder =================
        memT = memT_holder[0]
        xT_down = mid.tile([128, NK, TOK], BF16, tag="xT_down")
        with nc.named_scope("D_dec"), tc.tile_pool(name="pd", bufs=1) as pw:
            causal = pw.tile([128, 8, TOK], BF16, tag="causal")
            nc.sync.dma_start(causal[:], t_causal[:].rearrange("(c p) q -> p c q", p=128))
            transpose_into(xT_down, dec_x, 0, 4)
            kv_dec = y0T
            for l in range(LD):
                w_sb = pw.tile([128, NK, 4 * D], BF16, tag="wqkv")
                nc.sync.dma_start(w_sb[:], t_saW[l].rearrange("(k p) n -> p k n", p=128))
                wo_sb = pw.tile([64, NH, D], BF16, tag="wo")
                nc.sync.dma_start(
                    wo_sb[:], t_saW[l, :, 3 * D:4 * D].rearrange("(h p) n -> p h n", p=64))
                cw_sb = pw.tile([128, NK, 4 * D], BF16, tag="wca")
                nc.sync.dma_start(cw_sb[:], t_caW[l].rearrange("(k p) n -> p k n", p=128))
                cwo_sb = pw.tile([64, NH, D], BF16, tag="wo2")
                nc.sync.dma_start(
                    cwo_sb[:], t_caW[l, :, 3 * D:4 * D].rearrange("(h p) n -> p h n", p=64))
                w1_sb = pw.tile([128, NK, FFD], BF16, tag="w1")
                nc.sync.dma_start(w1_sb[:], t_decW1[l].rearrange("(k p) n -> p k n", p=128))
                w2_sb = pw.tile([128, NKF, D], BF16, tag="w2")
                nc.sync.dma_start(w2_sb[:], t_decW2[l].rearrange("(k p) n -> p k n", p=128))

                s1s = attention(pw, 2 + l, xT_down, TOK, kv_dec, T, w_sb, wo_sb,
                                tmln, causal, dec_x, dec_nat_b, "s1a")
                xT_sa = pw.tile([128, NK, TOK], BF16, tag="xT_dmid")
                layer_norm(pw, dec_nat_b, 4, 4 + 3 * l, s1s, xT_sa)
                s1s = attention(pw, 4 + l, xT_sa, TOK, memT, P, cw_sb, cwo_sb,
                                pmln, None, dec_nat_b, dec_nat_c, "s1c")
                xT_ca = pw.tile([128, NK, TOK], BF16, tag="xT_dmid")
                layer_norm(pw, dec_nat_c, 4, 4 + 3 * l + 1, s1s, xT_ca)
                s1s = ffn(pw, 2 + l, xT_ca, TOK, w1_sb, w2_sb, dec_nat_c, dec_x)
                layer_norm(pw, dec_x, 4, 4 + 3 * l + 2, s1s, xT_down,
                           natural=(l == 0 or debug))
                if l == 0:
                    kv_dec = pair_ag(mid, xT_down, TOK, "ag3", dst_tag="y0T")
                    if debug:
                        nc.sync.dma_start(
                            dbg['dec0'][:].rearrange("(g p) n -> p g n", p=128), dec_x[:])
            if debug:
                nc.sync.dma_start(
                    dbg['dec1'][:].rearrange("(g p) n -> p g n", p=128), dec_x[:])
            # kick off the final 8-rank AllGather while pd closes
            fi = dram.tile([NK * 128, TOK], BF16, tag="fag_i")
            nc.sync.dma_start(fi.opt().rearrange("(k p) t -> p k t", p=128), xT_down[:])
            fo = dram.tile([N_CORES * NK * 128, TOK], BF16, tag="fag_o",
                           addr_space="Shared")
            nc.gpsimd.collective_compute(
                "AllGather", OP.bypass, replica_groups=ALL8,
                ins=[fi.opt()], outs=[fo.opt()])

        # release stage A-D PSUM pools so stage E can rotate over 8 banks
        ps_stack.close()

        # ================= stage E: final projection =================
        with nc.named_scope("E_proj"), \
             tc.tile_pool(name="pf", bufs=1) as pw, \
             tc.tile_pool(name="plg", bufs=6) as plg, \
             tc.tile_pool(name="ps_e", bufs=8, space="PSUM") as ps_e:
            wout_sb = pw.tile([128, NK, VS], BF16, tag="wout")
            nc.sync.dma_start(wout_sb[:], t_wout[:].rearrange("(k p) n -> p k n", p=128))
            boutb = pw.tile([128, VS], FP32, tag="boutb")
            nc.sync.dma_start(boutb[:], t_boutb[:])
            yT_all = pw.tile([128, NK, B * T], BF16, tag="yT_all")
            for r in range(N_CORES):
                nc.sync.dma_start(
                    yT_all[:, :, r * TOK:(r + 1) * TOK],
                    fo.opt()[r * NK * 128:(r + 1) * NK * 128]
                    .rearrange("(k p) t -> p k t", p=128))
            for vg in range(NVT // 4):           # 2 groups of 4 vocab tiles
                for tg in range(B * T // 128):
                    pps = [ps_e.tile([128, 512], FP32, tag="e",
                                     name=f"pe_{vg}_{tg}_{j}") for j in range(4)]
                    for kk in range(NK):
                        for j in range(4):
                            vt = vg * 4 + j
                            nc.tensor.matmul(
                                pps[j][:, :VT],
                                yT_all[:, kk, tg * 128:(tg + 1) * 128],
                                wout_sb[:, kk, vt * VT:(vt + 1) * VT],
                                start=(kk == 0), stop=(kk == NK - 1))
                    lg = plg.tile([128, 4, VT], BF16, tag="lg")
                    for j in range(4):
                        vt = vg * 4 + j
                        nc.vector.tensor_tensor(
                            out=lg[:, j, :], in0=pps[j][:, :VT],
                            in1=boutb[:, vt * VT:(vt + 1) * VT], op=OP.add)
                    nc.sync.dma_start(
                        t_out[vg * 4:(vg + 1) * 4, tg].rearrange("v p n -> p v n"),
                        lg[:])

    nc.compile()
    return nc


# ---------------------------------------------------------------------------
# host side
# ---------------------------------------------------------------------------

def _bf(x):
    return np.ascontiguousarray(np.asarray(x, np.float32)).astype(BF)


def _f32(x):
    return np.ascontiguousarray(np.asarray(x, np.float32))


def _prep_inputs(inputs):
    ngram_ids = np.asarray(inputs['ngram_ids'])
    patch_mask = np.asarray(inputs['patch_mask'])
    target_ids = np.asarray(inputs['target_ids'])
    target_mask = np.asarray(inputs['target_mask'])
    tables = _f32(inputs['tables'])
    patch_pos = _f32(inputs['patch_pos'])
    token_emb = _f32(inputs['token_emb'])
    token_pos = _f32(inputs['token_pos'])
    enc_W = _f32(inputs['enc_W']); enc_b = _f32(inputs['enc_b'])
    enc_W1 = _f32(inputs['enc_W1']); enc_b1 = _f32(inputs['enc_b1'])
    enc_W2 = _f32(inputs['enc_W2']); enc_b2 = _f32(inputs['enc_b2'])
    enc_lng = _f32(inputs['enc_lng']); enc_lnb = _f32(inputs['enc_lnb'])
    dec_saW = _f32(inputs['dec_saW']); dec_sab = _f32(inputs['dec_sab'])
    dec_caW = _f32(inputs['dec_caW']); dec_cab = _f32(inputs['dec_cab'])
    dec_W1 = _f32(inputs['dec_W1']); dec_b1 = _f32(inputs['dec_b1'])
    dec_W2 = _f32(inputs['dec_W2']); dec_b2 = _f32(inputs['dec_b2'])
    dec_lng = _f32(inputs['dec_lng']); dec_lnb = _f32(inputs['dec_lnb'])
    Wout = _f32(inputs['Wout']); bout = _f32(inputs['bout'])

    stacked = _bf(tables.reshape(3 * BUCKETS, D))
    temb_bf = _bf(token_emb)
    tposT_bf = _bf(token_pos[:T].T)

    encWp = np.stack([_bf(np.concatenate([enc_W[l, i] for i in range(4)], axis=1))
                      for l in range(LE)])
    saWp = np.stack([_bf(np.concatenate([dec_saW[l, i] for i in range(4)], axis=1))
                     for l in range(LD)])
    caWp = np.stack([_bf(np.concatenate([dec_caW[l, i] for i in range(4)], axis=1))
                     for l in range(LD)])
    encW1b = _bf(enc_W1); encW2b = _bf(enc_W2)
    decW1b = _bf(dec_W1); decW2b = _bf(dec_W2)

    inst_Wb = [(enc_W[0], enc_b[0]), (enc_W[1], enc_b[1]),
               (dec_saW[0], dec_sab[0]), (dec_saW[1], dec_sab[1]),
               (dec_caW[0], dec_cab[0]), (dec_caW[1], dec_cab[1])]
    qkvb = np.zeros((6, 3, NK, 128), np.float32)
    ob_bc = np.zeros((6, 128, D), np.float32)
    ob_eff_l = []
    for a, (W4, b4) in enumerate(inst_Wb):
        qkvb[a] = b4[0:3].reshape(3, NK, 128)
        ob_eff = b4[3] + b4[2] @ W4[3]
        ob_eff_l.append(ob_eff)
        ob_bc[a] = np.broadcast_to(ob_eff[None, :], (128, D))
    b1pp = np.stack([enc_b1[0], enc_b1[1], dec_b1[0], dec_b1[1]]).reshape(4, NKF, 128)
    b2_l = [enc_b2[0], enc_b2[1], dec_b2[0], dec_b2[1]]
    b2_bc = np.stack([np.broadcast_to(v[None, :], (128, D)) for v in b2_l])
    ln_list = [enc_lng[0, 0], enc_lng[0, 1], enc_lng[1, 0], enc_lng[1, 1],
               dec_lng[0, 0], dec_lng[0, 1], dec_lng[0, 2],
               dec_lng[1, 0], dec_lng[1, 1], dec_lng[1, 2]]
    lnb_list = [enc_lnb[0, 0], enc_lnb[0, 1], enc_lnb[1, 0], enc_lnb[1, 1],
                dec_lnb[0, 0], dec_lnb[0, 1], dec_lnb[0, 2],
                dec_lnb[1, 0], dec_lnb[1, 1], dec_lnb[1, 2]]
    ln_g_bc = np.stack([np.broadcast_to(v[None, :], (128, D)) for v in ln_list])
    ln_b_bc = np.stack([np.broadcast_to(v[None, :], (128, D)) for v in lnb_list])
    ln_gT = np.ascontiguousarray(
        np.stack([v.reshape(NK, 128).T for v in ln_list])).astype(np.float32)
    ln_bT = np.ascontiguousarray(
        np.stack([v.reshape(NK, 128).T for v in lnb_list])).astype(np.float32)

    tril = np.tril(np.ones((T, T), np.float32))

    in_maps = []
    for c in range(N_CORES):
        b = c // 2
        h = c % 2
        ng = ngram_ids[b, h * PP:(h + 1) * PP].astype(np.int64)
        ng = ng + (np.arange(3) * BUCKETS)[None, :, None]
        ng = np.ascontiguousarray(ng.reshape(PP, 3 * K)).astype(np.int32)
        pm_own = patch_mask[b, h * PP:(h + 1) * PP].astype(np.float32)[:, None]
        pm_ln = np.where(patch_mask[b].astype(bool), 0.0, NEG).astype(np.float32)
        pm_ln = np.ascontiguousarray(pm_ln.reshape(2, 128).T)
        tm_ln = np.where(target_mask[b].astype(bool), 0.0, NEG).astype(np.float32)
        tm_ln = np.ascontiguousarray(tm_ln.reshape(8, 128).T)
        tids_all = np.ascontiguousarray(
            target_ids[b].reshape(8, 128).T).astype(np.int32)
        tids_own = np.ascontiguousarray(
            target_ids[b, h * TOK:(h + 1) * TOK].reshape(4, 128).T).astype(np.int32)
        causal01 = _bf(tril[h * TOK:(h + 1) * TOK, :].T)
        in_maps.append({
            "tables_st": stacked,
            "ng_ids": ng,
            "ppos_own": np.ascontiguousarray(patch_pos[h * PP:(h + 1) * PP]),
            "pmask_own": np.ascontiguousarray(pm_own),
            "pmask_ln": pm_ln,
            "token_emb": temb_bf,
            "tids_all": tids_all,
            "tids_own": tids_own,
            "tposT": tposT_bf,
            "tpos_own": np.ascontiguousarray(token_pos[h * TOK:(h + 1) * TOK]),
            "tmask_ln": tm_ln,
            "causal01": causal01,
            "enc_Wp": encWp,
            "enc_W1": encW1b,
            "enc_W2": encW2b,
            "dec_saWp": saWp,
            "dec_caWp": caWp,
            "dec_W1": decW1b,
            "dec_W2": decW2b,
            "wout": _bf(Wout[:, c * VS:(c + 1) * VS]),
            "qkvb": qkvb,
            "ob_bc": ob_bc,
            "b1pp": b1pp,
            "b2_bc": b2_bc,
            "ln_g_bc": ln_g_bc,
            "ln_b_bc": ln_b_bc,
            "ln_gT": ln_gT,
            "ln_bT": ln_bT,
            "bout_bc": np.ascontiguousarray(
                np.broadcast_to(bout[None, c * VS:(c + 1) * VS], (128, VS))).astype(np.float32),
        })
    return in_maps


def run(inputs, debug=False, trace=False):
    key = ("dbg" if debug else "rel")
    if key not in _CACHE:
        _CACHE[key] = build_program(debug=debug)
    nc = _CACHE[key]
    in_maps = _prep_inputs(inputs)
    res = run_bass_kernel_spmd(nc, in_maps, core_ids=list(range(N_CORES)),
                               trace=trace)
    return res


def assemble(res):
    out = np.zeros((B * T, VOCAB), np.float32)
    for c in range(N_CORES):
        lg = res.results[c]["logits"]          # [NVT, 32, 128, VT] bf16
        lg = np.asarray(lg, np.float32).transpose(1, 2, 0, 3).reshape(B * T, VS)
        out[:, c * VS:(c + 1) * VS] = lg
    return out.reshape(B, T, VOCAB)


def kernel(**inputs):
    return assemble(run(inputs))


# revision 31
# speedup vs baseline: 1.0020x; 1.0020x over previous
"""Trainium2 Bass kernel for nn_BLTModel (BLT encoder-decoder), 8 NeuronCores.

Core c -> batch b=c//2, token-half h=c%2. Patch-embed gathers + encoder +
decoder are data-parallel over batch (4 pairs) with a token split inside each
pair; per-pair bf16 AllGathers exchange activations. The final projection is
vocab-sharded: one 8-rank AllGather of the decoder output, then each core
computes logits [4096, 4000] bf16 for its vocab slice and the host
concatenates. Activations are kept transposed (xT [D, tok]) so every linear
is a direct PE matmul; LayerNorm runs in natural layout with the row-sum
fused into the residual add (scalar_tensor_tensor accum) and the rest on the
Scalar engine (Square/Rsqrt/scaled-copy). Attention softmax: exp with key
mask folded as a log-bias, causal mask as 0/1 multiply, denominators via a
ones column appended to V, normalized after the head loop with one batched
ACT reciprocal + gpsimd partition broadcasts. Engine balance: PE matmuls,
ACT exp/LN, DVE evac/adds, GpSimd bias adds + broadcasts.
"""
import contextlib
import sys

if '/opt/trn_rl_repo' not in sys.path:
    sys.path.insert(0, '/opt/trn_rl_repo')

import numpy as np
import ml_dtypes

import concourse.bass as bass
import concourse.mybir as mybir
import concourse.tile as tile
from concourse import bacc
from concourse.bass_utils import run_bass_kernel_spmd
from concourse.masks import make_identity

FP32 = mybir.dt.float32
BF16 = mybir.dt.bfloat16
I32 = mybir.dt.int32
BF = ml_dtypes.bfloat16
AF = mybir.ActivationFunctionType
OP = mybir.AluOpType
AX = mybir.AxisListType

B, P, T, K = 4, 256, 1024, 8
D, FFD, NH = 512, 2048, 8
LE, LD = 2, 2
VOCAB, BUCKETS = 32000, 50000
HD = D // NH
N_CORES = 8
PP = P // 2
TOK = T // 2
NK = D // 128
NKF = FFD // 128
VS = VOCAB // N_CORES
VT = 500
NVT = VS // VT
NEG = -1e9
EPS = 1e-5
PAIRS = [[2 * i, 2 * i + 1] for i in range(4)]
ALL8 = [list(range(N_CORES))]

_CACHE = {}


def build_program(debug=False):
    nc = bacc.Bacc("TRN2", target_bir_lowering=False, debug=False,
                   num_devices=N_CORES)

    # ---- inputs ----
    t_tables = nc.dram_tensor("tables_st", [3 * BUCKETS, D], BF16, kind="ExternalInput")
    t_ngids = nc.dram_tensor("ng_ids", [PP, 3 * K], I32, kind="ExternalInput")
    t_ppos = nc.dram_tensor("ppos_own", [PP, D], FP32, kind="ExternalInput")
    t_pmask = nc.dram_tensor("pmask_own", [PP, 1], FP32, kind="ExternalInput")
    t_pmln = nc.dram_tensor("pmask_ln", [128, 2], FP32, kind="ExternalInput")
    t_temb = nc.dram_tensor("token_emb", [VOCAB, D], BF16, kind="ExternalInput")
    t_tall = nc.dram_tensor("tids_all", [128, 8], I32, kind="ExternalInput")
    t_town = nc.dram_tensor("tids_own", [128, 4], I32, kind="ExternalInput")
    t_tposT = nc.dram_tensor("tposT", [D, T], BF16, kind="ExternalInput")
    t_tpos_own = nc.dram_tensor("tpos_own", [TOK, D], FP32, kind="ExternalInput")
    t_tmln = nc.dram_tensor("tmask_ln", [128, 8], FP32, kind="ExternalInput")
    t_causal = nc.dram_tensor("causal01", [T, TOK], BF16, kind="ExternalInput")

    t_encW = nc.dram_tensor("enc_Wp", [LE, D, 4 * D], BF16, kind="ExternalInput")
    t_encW1 = nc.dram_tensor("enc_W1", [LE, D, FFD], BF16, kind="ExternalInput")
    t_encW2 = nc.dram_tensor("enc_W2", [LE, FFD, D], BF16, kind="ExternalInput")
    t_saW = nc.dram_tensor("dec_saWp", [LD, D, 4 * D], BF16, kind="ExternalInput")
    t_caW = nc.dram_tensor("dec_caWp", [LD, D, 4 * D], BF16, kind="ExternalInput")
    t_decW1 = nc.dram_tensor("dec_W1", [LD, D, FFD], BF16, kind="ExternalInput")
    t_decW2 = nc.dram_tensor("dec_W2", [LD, FFD, D], BF16, kind="ExternalInput")
    t_wout = nc.dram_tensor("wout", [D, VS], BF16, kind="ExternalInput")

    t_qkvb = nc.dram_tensor("qkvb", [6, 3, NK, 128], FP32, kind="ExternalInput")
    t_ob = nc.dram_tensor("ob_bc", [6, 128, D], FP32, kind="ExternalInput")
    t_b1 = nc.dram_tensor("b1pp", [4, NKF, 128], FP32, kind="ExternalInput")
    t_b2 = nc.dram_tensor("b2_bc", [4, 128, D], FP32, kind="ExternalInput")
    t_lng = nc.dram_tensor("ln_g_bc", [10, 128, D], FP32, kind="ExternalInput")
    t_lnb = nc.dram_tensor("ln_b_bc", [10, 128, D], FP32, kind="ExternalInput")
    t_lngT = nc.dram_tensor("ln_gT", [10, 128, NK], FP32, kind="ExternalInput")
    t_lnbT = nc.dram_tensor("ln_bT", [10, 128, NK], FP32, kind="ExternalInput")
    t_boutb = nc.dram_tensor("bout_bc", [128, VS], FP32, kind="ExternalInput")

    t_out = nc.dram_tensor("logits", [NVT, B * T // 128, 128, VT], BF16, kind="ExternalOutput")
    dbg = {}
    if debug:
        dbg['pe'] = nc.dram_tensor("d_pe", [PP, D], FP32, kind="ExternalOutput")
        dbg['enc0'] = nc.dram_tensor("d_enc0", [PP, D], FP32, kind="ExternalOutput")
        dbg['mem'] = nc.dram_tensor("d_mem", [PP, D], FP32, kind="ExternalOutput")
        dbg['y0'] = nc.dram_tensor("d_y0", [TOK, D], FP32, kind="ExternalOutput")
        dbg['dec0'] = nc.dram_tensor("d_dec0", [TOK, D], FP32, kind="ExternalOutput")
        dbg['dec1'] = nc.dram_tensor("d_dec1", [TOK, D], FP32, kind="ExternalOutput")

    ps_stack = contextlib.ExitStack()

    with tile.TileContext(nc) as tc, \
         tc.tile_pool(name="const", bufs=1) as const, \
         tc.tile_pool(name="sml", bufs=2) as sml, \
         tc.tile_pool(name="mid", bufs=1) as mid, \
         tc.tile_pool(name="dram", bufs=1, space="DRAM") as dram:

        ps_lin = ps_stack.enter_context(tc.tile_pool(name="ps_lin", bufs=2, space="PSUM"))
        ps_sc = ps_stack.enter_context(tc.tile_pool(name="ps_sc", bufs=2, space="PSUM"))
        ps_z = ps_stack.enter_context(tc.tile_pool(name="ps_z", bufs=2, space="PSUM"))
        ps_tp = ps_stack.enter_context(tc.tile_pool(name="ps_tp", bufs=2, space="PSUM"))

        # ================= constants (small, whole-program) =================
        ident = const.tile([128, 128], FP32)
        make_identity(nc, ident[:])
        identb = const.tile([128, 128], BF16)
        make_identity(nc, identb[:])
        epsc = const.tile([128, 1], FP32)
        nc.vector.memset(epsc[:], EPS)
        pmln = const.tile([128, 2], FP32)
        nc.sync.dma_start(pmln[:], t_pmln[:])
        tmln = const.tile([128, 8], FP32)
        nc.sync.dma_start(tmln[:], t_tmln[:])
        qkvb = const.tile([128, 6, 3, NK], FP32)
        nc.sync.dma_start(qkvb[:], t_qkvb[:].rearrange("a m g p -> p a m g"))
        b1c = const.tile([128, 4, NKF], FP32)
        nc.sync.dma_start(b1c[:], t_b1[:].rearrange("a g p -> p a g"))
        lngT = const.tile([128, 10, NK], FP32)
        nc.sync.dma_start(lngT[:], t_lngT[:].rearrange("i p k -> p i k"))
        lnbT = const.tile([128, 10, NK], FP32)
        nc.sync.dma_start(lnbT[:], t_lnbT[:].rearrange("i p k -> p i k"))
        ones1r = const.tile([1, 128], FP32)
        nc.vector.memset(ones1r[:], 1.0)

        # ============ persistent activations (cross-phase) ============
        pe_nat = mid.tile([128, 1, D], FP32, tag="enc_nat_a")
        enc_nat_b = mid.tile([128, 1, D], FP32, tag="enc_nat_b")
        dec_x = mid.tile([128, 4, D], FP32, tag="dec_nat_a")
        dec_nat_b = mid.tile([128, 4, D], FP32, tag="dec_nat_b")
        dec_nat_c = mid.tile([128, 4, D], FP32, tag="dec_nat_c")
        y0T = mid.tile([128, NK, T], BF16, tag="y0T")
        memT_holder = []

        # ================= helpers =================
        def transpose_into(xT_dst, src_nat, tg0, ngrp, src_bf=False):
            idt = identb if src_bf else ident
            lp = (nc.allow_low_precision(reason="bf16 transpose, no accumulation")
                  if src_bf else contextlib.nullcontext())
            with lp:
                i = 0
                for g in range(ngrp):
                    for kk in range(NK):
                        tp = ps_tp.tile([128, 128], BF16 if src_bf else FP32, tag="tp")
                        nc.tensor.transpose(
                            tp[:], src_nat[:, g, kk * 128:(kk + 1) * 128], idt[:])
                        dst = xT_dst[:, kk, (tg0 + g) * 128:(tg0 + g + 1) * 128]
                        if i % 2 == 0:
                            nc.scalar.copy(dst, tp[:])
                        else:
                            nc.vector.tensor_copy(out=dst, in_=tp[:])
                        i += 1

        def layer_norm(pool, sum_nat, ngrp, ln_i, s1s, xT_dst, natural=True):
            """Fused LN + transpose. Writes normalized, affine-transformed,
            transposed output to xT_dst (bf16). The per-token 1/sigma scaling
            rides the transpose matmul as a diag(rs) rhs; the per-feature g/b
            affine rides the PSUM evacuation. If `natural`, also materializes
            the LN output in sum_nat (needed when it feeds a residual)."""
            if natural:
                lg_t = pool.tile([128, D], FP32, tag="lng")
                nc.sync.dma_start(lg_t[:], t_lng[ln_i])
                lb_t = pool.tile([128, D], FP32, tag="lnb")
                nc.sync.dma_start(lb_t[:], t_lnb[ln_i])
            for g in range(ngrp):
                x = sum_nat[:, g, :]
                negmu = sml.tile([128, 1], FP32, tag="ln_nm", bufs=4)
                nc.scalar.mul(negmu[:], s1s[g][:], -1.0 / D)
                sq = sml.tile([128, D], FP32, tag="ln_sq", bufs=1)
                ss = sml.tile([128, 1], FP32, tag="ln_ss", bufs=4)
                nc.scalar.activation(sq[:], x, AF.Square, bias=negmu[:, :1],
                                     accum_out=ss[:, :1])
                sd = sml.tile([128, 1], FP32, tag="ln_sd", bufs=4)
                nc.scalar.activation(sd[:], ss[:], AF.Sqrt, bias=epsc[:, :1],
                                     scale=1.0 / D)
                rs = sml.tile([128, 1], FP32, tag="ln_rs", bufs=4)
                nc.vector.reciprocal(rs[:], sd[:])
                xc = sml.tile([128, D], FP32, tag="ln_xc", bufs=2)
                nc.vector.tensor_scalar(out=xc[:], in0=x, scalar1=negmu[:, :1],
                                        scalar2=None, op0=OP.add)
                diag = sml.tile([128, 128], FP32, tag="ln_diag", bufs=2)
                nc.vector.tensor_scalar(out=diag[:], in0=ident[:],
                                        scalar1=rs[:, :1], scalar2=None,
                                        op0=OP.mult)
                for kk in range(NK):
                    tp = ps_tp.tile([128, 128], FP32, tag="tp")
                    nc.tensor.matmul(tp[:], xc[:, kk * 128:(kk + 1) * 128],
                                     diag[:], start=True, stop=True)
                    nc.vector.tensor_scalar(
                        out=xT_dst[:, kk, g * 128:(g + 1) * 128], in0=tp[:],
                        scalar1=lngT[:, ln_i, kk:kk + 1],
                        scalar2=lnbT[:, ln_i, kk:kk + 1],
                        op0=OP.mult, op1=OP.add)
                if natural:
                    nc.vector.tensor_scalar(out=x, in0=xc[:], scalar1=rs[:, :1],
                                            scalar2=None, op0=OP.mult)
                    nc.vector.tensor_tensor(out=x, in0=x, in1=lg_t[:], op=OP.mult)
                    nc.vector.tensor_tensor(out=x, in0=x, in1=lb_t[:], op=OP.add)

        def attention(pool, a_i, xT_q, nq, xT_kv, nkv, w_sb, wo_sb, mask_ln,
                      causal_sb, resid_nat, out_sum, s1_tag):
            nkc = nkv // 128
            ntg = nq // 128
            qT = pool.tile([128, NK, nq], BF16, tag="qT")
            kT = pool.tile([128, NK, nkv], BF16, tag="kT")
            for m, dstT, src, ncols in ((0, qT, xT_q, nq), (1, kT, xT_kv, nkv)):
                for g in range(NK):
                    for c0 in range(0, ncols, 512):
                        cw = min(512, ncols - c0)
                        pp = ps_lin.tile([128, 512], FP32, tag="lin")
                        for kk in range(NK):
                            nc.tensor.matmul(
                                pp[:, :cw],
                                w_sb[:, kk, m * D + g * 128: m * D + (g + 1) * 128],
                                src[:, kk, c0:c0 + cw],
                                start=(kk == 0), stop=(kk == NK - 1))
                        nc.vector.tensor_scalar(
                            out=dstT[:, g, c0:c0 + cw], in0=pp[:, :cw],
                            scalar1=qkvb[:, a_i, m, g:g + 1], scalar2=None,
                            op0=OP.add)
            v_sb = pool.tile([128, nkc, NH, HD + 1], BF16, tag="v")
            nc.vector.memset(v_sb[:, :, :, HD:HD + 1], 1.0)
            for c in range(nkc):
                pp = ps_lin.tile([128, 512], FP32, tag="lin")
                for kk in range(NK):
                    nc.tensor.matmul(
                        pp[:], xT_kv[:, kk, c * 128:(c + 1) * 128],
                        w_sb[:, kk, 2 * D:3 * D],
                        start=(kk == 0), stop=(kk == NK - 1))
                nc.vector.tensor_copy(
                    out=v_sb[:, c, :, :HD],
                    in_=pp[:].rearrange("p (h d) -> p h d", h=NH))
            ob_t = pool.tile([128, D], FP32, tag="ob")
            nc.sync.dma_start(ob_t[:], t_ob[a_i])
            zT = pool.tile([64, NH, nq], BF16, tag="zT")
            for hp in range(NH // 2):
                zps = []
                for s in range(2):
                    zp = ps_z.tile([128, 512], FP32, tag="z",
                                   name=f"zp_{a_i}_{hp}_{s}")
                    zps.append(zp)
                for c in range(nkc):
                    sps = []
                    for s in range(2):
                        h = 2 * hp + s
                        pl = (h % 2) * 64
                        gq = h // 2
                        sp = ps_sc.tile([128, 512], FP32, tag="sc",
                                        name=f"sp_{a_i}_{hp}_{c}_{s}")
                        nc.tensor.matmul(
                            sp[:, :nq],
                            kT[pl:pl + 64, gq, c * 128:(c + 1) * 128],
                            qT[pl:pl + 64, gq, :nq],
                            start=True, stop=True)
                        sps.append(sp)
                    for s in range(2):
                        h = 2 * hp + s
                        a_sb = sml.tile([128, 512], BF16, tag="a", bufs=4)
                        nc.scalar.activation(a_sb[:, :nq], sps[s][:, :nq], AF.Exp,
                                             bias=mask_ln[:, c:c + 1],
                                             scale=float(1.0 / np.sqrt(HD)))
                        if causal_sb is not None:
                            nc.vector.tensor_tensor(
                                out=a_sb[:, :nq], in0=a_sb[:, :nq],
                                in1=causal_sb[:, c, :nq], op=OP.mult)
                        nc.tensor.matmul(
                            zps[s][:HD + 1, :nq], v_sb[:, c, h, :], a_sb[:, :nq],
                            start=(c == 0), stop=(c == nkc - 1))
                for s in range(2):
                    h = 2 * hp + s
                    rcin = sml.tile([1, 512], FP32, tag="rcin", bufs=1)
                    nc.scalar.copy(rcin[:, :nq], zps[s][HD:HD + 1, :nq])
                    rc = sml.tile([1, 512], FP32, tag="rc", bufs=2)
                    nc.vector.reciprocal_approx_fast(
                        out=rc[:, :nq], in_=rcin[:, :nq])
                    bcd = pool.tile([64, 512], FP32, tag="bcd", bufs=2)
                    nc.gpsimd.partition_broadcast(bcd[:, :nq], rc[:, :nq])
                    with nc.allow_low_precision(reason="softmax denom recip"):
                        nc.vector.tensor_tensor(out=zT[:, h, :nq],
                                                in0=zps[s][:HD, :nq],
                                                in1=bcd[:, :nq], op=OP.mult)
            s1s = []
            for g in range(ntg):
                op_ = ps_lin.tile([128, 512], FP32, tag="lin")
                for h in range(NH):
                    nc.tensor.matmul(op_[:], zT[:, h, g * 128:(g + 1) * 128],
                                     wo_sb[:, h, :],
                                     start=(h == 0), stop=False)
                nc.tensor.matmul(op_[:], ones1r[:, :], ob_t[0:1, :],
                                 start=False, stop=True)
                s1 = sml.tile([128, 1], FP32, tag=s1_tag, bufs=6)
                nc.vector.scalar_tensor_tensor(
                    out=out_sum[:, g, :], in0=op_[:], scalar=0.0,
                    in1=resid_nat[:, g, :], op0=OP.bypass, op1=OP.add,
                    accum_out=s1[:, :1])
                s1s.append(s1)
            return s1s

        def ffn(pool, ff_i, xT, ntok, w1_sb, w2_sb, resid_nat, out_sum):
            b2_t = pool.tile([128, D], FP32, tag="b2")
            nc.sync.dma_start(b2_t[:], t_b2[ff_i])
            th = min(ntok, 256)
            s1s = []
            for t0 in range(0, ntok, th):
                hT = pool.tile([128, NKF, th], BF16, tag="hT")
                for fg in range(NKF):
                    pp = ps_lin.tile([128, 512], FP32, tag="lin")
                    for kk in range(NK):
                        nc.tensor.matmul(pp[:, :th],
                                         w1_sb[:, kk, fg * 128:(fg + 1) * 128],
                                         xT[:, kk, t0:t0 + th],
                                         start=(kk == 0), stop=(kk == NK - 1))
                    nc.scalar.activation(hT[:, fg, :], pp[:, :th], AF.Relu,
                                         bias=b1c[:, ff_i, fg:fg + 1], scale=1.0)
                for g in range(th // 128):
                    gg = t0 // 128 + g
                    pp = ps_lin.tile([128, 512], FP32, tag="lin")
                    for fg in range(NKF):
                        nc.tensor.matmul(pp[:], hT[:, fg, g * 128:(g + 1) * 128],
                                         w2_sb[:, fg, :],
                                         start=(fg == 0), stop=False)
                    nc.tensor.matmul(pp[:], ones1r[:, :], b2_t[0:1, :],
                                     start=False, stop=True)
                    s1 = sml.tile([128, 1], FP32, tag="s1f", bufs=6)
                    nc.vector.scalar_tensor_tensor(
                        out=out_sum[:, gg, :], in0=pp[:], scalar=0.0,
                        in1=resid_nat[:, gg, :], op0=OP.bypass, op1=OP.add,
                        accum_out=s1[:, :1])
                    s1s.append(s1)
            return s1s

        def pair_ag(dstpool, src_sb, ncols, tag, dst_tag=None):
            bi = dram.tile([NK * 128, ncols], BF16, tag=tag + "_i")
            nc.sync.dma_start(
                bi.opt().rearrange("(k p) t -> p k t", p=128), src_sb[:])
            bo = dram.tile([2 * NK * 128, ncols], BF16, tag=tag + "_o")
            nc.gpsimd.collective_compute(
                "AllGather", OP.bypass, replica_groups=PAIRS,
                ins=[bi.opt()], outs=[bo.opt()])
            dst = dstpool.tile([128, NK, 2 * ncols], BF16,
                               tag=(dst_tag or (tag + "_d")))
            for r in range(2):
                nc.sync.dma_start(
                    dst[:, :, r * ncols:(r + 1) * ncols],
                    bo.opt()[r * NK * 128:(r + 1) * NK * 128]
                    .rearrange("(k p) t -> p k t", p=128))
            return dst

        # ================= stage A: patch embedding =================
        xT_eown = mid.tile([128, NK, PP], BF16, tag="xT_eown")
        with nc.named_scope("A_patch"), tc.tile_pool(name="pa", bufs=1) as pa:
            ngid = pa.tile([PP, 3 * K], I32)
            nc.sync.dma_start(ngid[:], t_ngids[:])
            gth = pa.tile([128, 3 * K, D], BF16, tag="gth")
            for k in range(3 * K):
                nc.gpsimd.indirect_dma_start(
                    out=gth[:, k, :], out_offset=None, in_=t_tables[:],
                    in_offset=bass.IndirectOffsetOnAxis(ap=ngid[:, k:k + 1], axis=0))
            acc0 = pa.tile([128, D], FP32, tag="pacc0")
            acc1 = pa.tile([128, D], FP32, tag="pacc1")
            nc.vector.tensor_tensor(out=acc0[:], in0=gth[:, 0, :], in1=gth[:, 2, :], op=OP.add)
            nc.vector.tensor_tensor(out=acc1[:], in0=gth[:, 1, :], in1=gth[:, 3, :], op=OP.add)
            for k in range(4, 3 * K, 2):
                nc.vector.tensor_tensor(out=acc0[:], in0=acc0[:], in1=gth[:, k, :], op=OP.add)
            for k in range(5, 3 * K, 2):
                nc.vector.tensor_tensor(out=acc1[:], in0=acc1[:], in1=gth[:, k, :], op=OP.add)
            nc.vector.tensor_tensor(out=pe_nat[:, 0, :], in0=acc0[:], in1=acc1[:], op=OP.add)
            pmask = pa.tile([PP, 1], FP32)
            nc.sync.dma_start(pmask[:], t_pmask[:])
            nc.vector.tensor_scalar(out=pe_nat[:, 0, :], in0=pe_nat[:, 0, :],
                                    scalar1=pmask[:, :1], scalar2=None, op0=OP.mult)
            ppos = pa.tile([PP, D], FP32, tag="ppos")
            nc.sync.dma_start(ppos[:], t_ppos[:])
            nc.vector.tensor_tensor(out=pe_nat[:, 0, :], in0=pe_nat[:, 0, :],
                                    in1=ppos[:], op=OP.add)
            if debug:
                nc.sync.dma_start(dbg['pe'][:], pe_nat[:, 0, :])
            transpose_into(xT_eown, pe_nat, 0, 1)

        # ================= stage B: encoder (+ token embedding overlap) ======
        with nc.named_scope("B_enc"), tc.tile_pool(name="pe_", bufs=1) as pw:
            xT_kv_enc = pair_ag(pw, xT_eown, PP, "ag0")

            # ---- token embedding (independent of encoder; fills AG hole) ----
            with nc.named_scope("C_tok"), tc.tile_pool(name="pc", bufs=1) as pc:
                tall = pc.tile([128, 8], I32)
                nc.sync.dma_start(tall[:], t_tall[:])
                town = pc.tile([128, 4], I32)
                nc.sync.dma_start(town[:], t_town[:])
                y0n = pc.tile([128, 8, D], BF16, tag="y0n")
                for c in range(8):
                    nc.gpsimd.indirect_dma_start(
                        out=y0n[:, c, :], out_offset=None, in_=t_temb[:],
                        in_offset=bass.IndirectOffsetOnAxis(ap=tall[:, c:c + 1], axis=0))
                transpose_into(y0T, y0n, 0, 8, src_bf=True)
                tposT = pc.tile([128, NK, T], BF16, tag="tposT")
                nc.sync.dma_start(tposT[:], t_tposT[:].rearrange("(k p) t -> p k t", p=128))
                nc.vector.tensor_tensor(out=y0T[:].rearrange("p k t -> p (k t)"),
                                        in0=y0T[:].rearrange("p k t -> p (k t)"),
                                        in1=tposT[:].rearrange("p k t -> p (k t)"), op=OP.add)
                y0o = pc.tile([128, 4, D], BF16, tag="y0o")
                for c in range(4):
                    nc.gpsimd.indirect_dma_start(
                        out=y0o[:, c, :], out_offset=None, in_=t_temb[:],
                        in_offset=bass.IndirectOffsetOnAxis(ap=town[:, c:c + 1], axis=0))
                tpos_o = pc.tile([128, 4, D], FP32, tag="tpos_o")
                nc.sync.dma_start(tpos_o[:], t_tpos_own[:].rearrange("(g p) n -> p g n", p=128))
                for g in range(4):
                    nc.vector.tensor_tensor(out=dec_x[:, g, :], in0=tpos_o[:, g, :],
                                            in1=y0o[:, g, :], op=OP.add)
                if debug:
                    nc.sync.dma_start(dbg['y0'][:].rearrange("(g p) n -> p g n", p=128),
                                      dec_x[:])

            for l in range(LE):
                w_sb = pw.tile([128, NK, 4 * D], BF16, tag="wqkv")
                nc.sync.dma_start(w_sb[:], t_encW[l].rearrange("(k p) n -> p k n", p=128))
                wo_sb = pw.tile([64, NH, D], BF16, tag="wo")
                nc.sync.dma_start(
                    wo_sb[:], t_encW[l, :, 3 * D:4 * D].rearrange("(h p) n -> p h n", p=64))
                w1_sb = pw.tile([128, NK, FFD], BF16, tag="w1")
                nc.sync.dma_start(w1_sb[:], t_encW1[l].rearrange("(k p) n -> p k n", p=128))
                w2_sb = pw.tile([128, NKF, D], BF16, tag="w2")
                nc.sync.dma_start(w2_sb[:], t_encW2[l].rearrange("(k p) n -> p k n", p=128))

                s1s = attention(pw, l, xT_eown, PP, xT_kv_enc, P, w_sb, wo_sb, pmln,
                                None, pe_nat, enc_nat_b, "s1a")
                xT_mid_t = pw.tile([128, NK, PP], BF16, tag="xT_emid")
                layer_norm(pw, enc_nat_b, 1, 2 * l, s1s, xT_mid_t)
                s1s = ffn(pw, l, xT_mid_t, PP, w1_sb, w2_sb, enc_nat_b, pe_nat)
                xT_eown = mid.tile([128, NK, PP], BF16, tag=f"xT_eo{l}")
                layer_norm(pw, pe_nat, 1, 2 * l + 1, s1s, xT_eown,
                           natural=(l == 0 or debug))
                if l == 0:
                    xT_kv_enc = pair_ag(pw, xT_eown, PP, "ag1")
                    if debug:
                        nc.sync.dma_start(dbg['enc0'][:], pe_nat[:, 0, :])
            memT = pair_ag(mid, xT_eown, PP, "ag2")
            memT_holder.append(memT)
            if debug:
                nc.sync.dma_start(dbg['mem'][:], pe_nat[:, 0, :])

        # ================= stage D: decoder =================
        memT = memT_holder[0]
        xT_down = mid.tile([128, NK, TOK], BF16, tag="xT_down")
        with nc.named_scope("D_dec"), tc.tile_pool(name="pd", bufs=1) as pw:
            causal = pw.tile([128, 8, TOK], BF16, tag="causal")
            nc.sync.dma_start(causal[:], t_causal[:].rearrange("(c p) q -> p c q", p=128))
            transpose_into(xT_down, dec_x, 0, 4)
            kv_dec = y0T
            for l in range(LD):
                w_sb = pw.tile([128, NK, 4 * D], BF16, tag="wqkv")
                nc.sync.dma_start(w_sb[:], t_saW[l].rearrange("(k p) n -> p k n", p=128))
                wo_sb = pw.tile([64, NH, D], BF16, tag="wo")
                nc.sync.dma_start(
                    wo_sb[:], t_saW[l, :, 3 * D:4 * D].rearrange("(h p) n -> p h n", p=64))
                cw_sb = pw.tile([128, NK, 4 * D], BF16, tag="wca")
                nc.sync.dma_start(cw_sb[:], t_caW[l].rearrange("(k p) n -> p k n", p=128))
                cwo_sb = pw.tile([64, NH, D], BF16, tag="wo2")
                nc.sync.dma_start(
                    cwo_sb[:], t_caW[l, :, 3 * D:4 * D].rearrange("(h p) n -> p h n", p=64))
                w1_sb = pw.tile([128, NK, FFD], BF16, tag="w1")
                nc.sync.dma_start(w1_sb[:], t_decW1[l].rearrange("(k p) n -> p k n", p=128))
                w2_sb = pw.tile([128, NKF, D], BF16, tag="w2")
                nc.sync.dma_start(w2_sb[:], t_decW2[l].rearrange("(k p) n -> p k n", p=128))

                s1s = attention(pw, 2 + l, xT_down, TOK, kv_dec, T, w_sb, wo_sb,
                                tmln, causal, dec_x, dec_nat_b, "s1a")
                xT_sa = pw.tile([128, NK, TOK], BF16, tag="xT_dmid")
                layer_norm(pw, dec_nat_b, 4, 4 + 3 * l, s1s, xT_sa)
                s1s = attention(pw, 4 + l, xT_sa, TOK, memT, P, cw_sb, cwo_sb,
                                pmln, None, dec_nat_b, dec_nat_c, "s1c")
                xT_ca = pw.tile([128, NK, TOK], BF16, tag="xT_dmid")
                layer_norm(pw, dec_nat_c, 4, 4 + 3 * l + 1, s1s, xT_ca)
                s1s = ffn(pw, 2 + l, xT_ca, TOK, w1_sb, w2_sb, dec_nat_c, dec_x)
                layer_norm(pw, dec_x, 4, 4 + 3 * l + 2, s1s, xT_down,
                           natural=(l == 0 or debug))
                if l == 0:
                    kv_dec = pair_ag(mid, xT_down, TOK, "ag3", dst_tag="y0T")
                    if debug:
                        nc.sync.dma_start(
                            dbg['dec0'][:].rearrange("(g p) n -> p g n", p=128), dec_x[:])
            if debug:
                nc.sync.dma_start(
                    dbg['dec1'][:].rearrange("(g p) n -> p g n", p=128), dec_x[:])
            # kick off the final 8-rank AllGather while pd closes
            fi = dram.tile([NK * 128, TOK], BF16, tag="fag_i")
            nc.sync.dma_start(fi.opt().rearrange("(k p) t -> p k t", p=128), xT_down[:])
            fo = dram.tile([N_CORES * NK * 128, TOK], BF16, tag="fag_o",
                           addr_space="Shared")
            nc.gpsimd.collective_compute(
                "AllGather", OP.bypass, replica_groups=ALL8,
                ins=[fi.opt()], outs=[fo.opt()])

        # release stage A-D PSUM pools so stage E can rotate over 8 banks
        ps_stack.close()

        # ================= stage E: final projection =================
        with nc.named_scope("E_proj"), \
             tc.tile_pool(name="pf", bufs=1) as pw, \
             tc.tile_pool(name="plg", bufs=6) as plg, \
             tc.tile_pool(name="ps_e", bufs=8, space="PSUM") as ps_e:
            wout_sb = pw.tile([128, NK, VS], BF16, tag="wout")
            nc.sync.dma_start(wout_sb[:], t_wout[:].rearrange("(k p) n -> p k n", p=128))
            boutb = pw.tile([128, VS], FP32, tag="boutb")
            nc.sync.dma_start(boutb[:], t_boutb[:])
            yT_all = pw.tile([128, NK, B * T], BF16, tag="yT_all")
            for r in range(N_CORES):
                nc.sync.dma_start(
                    yT_all[:, :, r * TOK:(r + 1) * TOK],
                    fo.opt()[r * NK * 128:(r + 1) * NK * 128]
                    .rearrange("(k p) t -> p k t", p=128))
            for vg in range(NVT // 4):           # 2 groups of 4 vocab tiles
                for tg in range(B * T // 128):
                    pps = [ps_e.tile([128, 512], FP32, tag="e",
                                     name=f"pe_{vg}_{tg}_{j}") for j in range(4)]
                    for kk in range(NK):
                        for j in range(4):
                            vt = vg * 4 + j
                            nc.tensor.matmul(
                                pps[j][:, :VT],
                                yT_all[:, kk, tg * 128:(tg + 1) * 128],
                                wout_sb[:, kk, vt * VT:(vt + 1) * VT],
                                start=(kk == 0), stop=(kk == NK - 1))
                    lg = plg.tile([128, 4, VT], BF16, tag="lg")
                    for j in range(4):
                        vt = vg * 4 + j
                        nc.vector.tensor_tensor(
                            out=lg[:, j, :], in0=pps[j][:, :VT],
                            in1=boutb[:, vt * VT:(vt + 1) * VT], op=OP.add)
                    nc.sync.dma_start(
                        t_out[vg * 4:(vg + 1) * 4, tg].rearrange("v p n -> p v n"),
                        lg[:])

    nc.compile()
    return nc


# ---------------------------------------------------------------------------
# host side
# ---------------------------------------------------------------------------

def _bf(x):
    return np.ascontiguousarray(np.asarray(x, np.float32)).astype(BF)


def _f32(x):
    return np.ascontiguousarray(np.asarray(x, np.float32))


def _prep_inputs(inputs):
    ngram_ids = np.asarray(inputs['ngram_ids'])
    patch_mask = np.asarray(inputs['patch_mask'])
    target_ids = np.asarray(inputs['target_ids'])
    target_mask = np.asarray(inputs['target_mask'])
    tables = _f32(inputs['tables'])
    patch_pos = _f32(inputs['patch_pos'])
    token_emb = _f32(inputs['token_emb'])
    token_pos = _f32(inputs['token_pos'])
    enc_W = _f32(inputs['enc_W']); enc_b = _f32(inputs['enc_b'])
    enc_W1 = _f32(inputs['enc_W1']); enc_b1 = _f32(inputs['enc_b1'])
    enc_W2 = _f32(inputs['enc_W2']); enc_b2 = _f32(inputs['enc_b2'])
    enc_lng = _f32(inputs['enc_lng']); enc_lnb = _f32(inputs['enc_lnb'])
    dec_saW = _f32(inputs['dec_saW']); dec_sab = _f32(inputs['dec_sab'])
    dec_caW = _f32(inputs['dec_caW']); dec_cab = _f32(inputs['dec_cab'])
    dec_W1 = _f32(inputs['dec_W1']); dec_b1 = _f32(inputs['dec_b1'])
    dec_W2 = _f32(inputs['dec_W2']); dec_b2 = _f32(inputs['dec_b2'])
    dec_lng = _f32(inputs['dec_lng']); dec_lnb = _f32(inputs['dec_lnb'])
    Wout = _f32(inputs['Wout']); bout = _f32(inputs['bout'])

    stacked = _bf(tables.reshape(3 * BUCKETS, D))
    temb_bf = _bf(token_emb)
    tposT_bf = _bf(token_pos[:T].T)

    encWp = np.stack([_bf(np.concatenate([enc_W[l, i] for i in range(4)], axis=1))
                      for l in range(LE)])
    saWp = np.stack([_bf(np.concatenate([dec_saW[l, i] for i in range(4)], axis=1))
                     for l in range(LD)])
    caWp = np.stack([_bf(np.concatenate([dec_caW[l, i] for i in range(4)], axis=1))
                     for l in range(LD)])
    encW1b = _bf(enc_W1); encW2b = _bf(enc_W2)
    decW1b = _bf(dec_W1); decW2b = _bf(dec_W2)

    inst_Wb = [(enc_W[0], enc_b[0]), (enc_W[1], enc_b[1]),
               (dec_saW[0], dec_sab[0]), (dec_saW[1], dec_sab[1]),
               (dec_caW[0], dec_cab[0]), (dec_caW[1], dec_cab[1])]
    qkvb = np.zeros((6, 3, NK, 128), np.float32)
    ob_bc = np.zeros((6, 128, D), np.float32)
    ob_eff_l = []
    for a, (W4, b4) in enumerate(inst_Wb):
        qkvb[a] = b4[0:3].reshape(3, NK, 128)
        ob_eff = b4[3] + b4[2] @ W4[3]
        ob_eff_l.append(ob_eff)
        ob_bc[a] = np.broadcast_to(ob_eff[None, :], (128, D))
    b1pp = np.stack([enc_b1[0], enc_b1[1], dec_b1[0], dec_b1[1]]).reshape(4, NKF, 128)
    b2_l = [enc_b2[0], enc_b2[1], dec_b2[0], dec_b2[1]]
    b2_bc = np.stack([np.broadcast_to(v[None, :], (128, D)) for v in b2_l])
    ln_list = [enc_lng[0, 0], enc_lng[0, 1], enc_lng[1, 0], enc_lng[1, 1],
               dec_lng[0, 0], dec_lng[0, 1], dec_lng[0, 2],
               dec_lng[1, 0], dec_lng[1, 1], dec_lng[1, 2]]
    lnb_list = [enc_lnb[0, 0], enc_lnb[0, 1], enc_lnb[1, 0], enc_lnb[1, 1],
                dec_lnb[0, 0], dec_lnb[0, 1], dec_lnb[0, 2],
                dec_lnb[1, 0], dec_lnb[1, 1], dec_lnb[1, 2]]
    ln_g_bc = np.stack([np.broadcast_to(v[None, :], (128, D)) for v in ln_list])
    ln_b_bc = np.stack([np.broadcast_to(v[None, :], (128, D)) for v in lnb_list])
    ln_gT = np.ascontiguousarray(
        np.stack([v.reshape(NK, 128).T for v in ln_list])).astype(np.float32)
    ln_bT = np.ascontiguousarray(
        np.stack([v.reshape(NK, 128).T for v in lnb_list])).astype(np.float32)

    tril = np.tril(np.ones((T, T), np.float32))

    in_maps = []
    for c in range(N_CORES):
        b = c // 2
        h = c % 2
        ng = ngram_ids[b, h * PP:(h + 1) * PP].astype(np.int64)
        ng = ng + (np.arange(3) * BUCKETS)[None, :, None]
        ng = np.ascontiguousarray(ng.reshape(PP, 3 * K)).astype(np.int32)
        pm_own = patch_mask[b, h * PP:(h + 1) * PP].astype(np.float32)[:, None]
        pm_ln = np.where(patch_mask[b].astype(bool), 0.0, NEG).astype(np.float32)
        pm_ln = np.ascontiguousarray(pm_ln.reshape(2, 128).T)
        tm_ln = np.where(target_mask[b].astype(bool), 0.0, NEG).astype(np.float32)
        tm_ln = np.ascontiguousarray(tm_ln.reshape(8, 128).T)
        tids_all = np.ascontiguousarray(
            target_ids[b].reshape(8, 128).T).astype(np.int32)
        tids_own = np.ascontiguousarray(
            target_ids[b, h * TOK:(h + 1) * TOK].reshape(4, 128).T).astype(np.int32)
        causal01 = _bf(tril[h * TOK:(h + 1) * TOK, :].T)
        in_maps.append({
            "tables_st": stacked,
            "ng_ids": ng,
            "ppos_own": np.ascontiguousarray(patch_pos[h * PP:(h + 1) * PP]),
            "pmask_own": np.ascontiguousarray(pm_own),
            "pmask_ln": pm_ln,
            "token_emb": temb_bf,
            "tids_all": tids_all,
            "tids_own": tids_own,
            "tposT": tposT_bf,
            "tpos_own": np.ascontiguousarray(token_pos[h * TOK:(h + 1) * TOK]),
            "tmask_ln": tm_ln,
            "causal01": causal01,
            "enc_Wp": encWp,
            "enc_W1": encW1b,
            "enc_W2": encW2b,
            "dec_saWp": saWp,
            "dec_caWp": caWp,
            "dec_W1": decW1b,
            "dec_W2": decW2b,
            "wout": _bf(Wout[:, c * VS:(c + 1) * VS]),
            "qkvb": qkvb,
            "ob_bc": ob_bc,
            "b1pp": b1pp,
            "b2_bc": b2_bc,
            "ln_g_bc": ln_g_bc,
            "ln_b_bc": ln_b_bc,
            "ln_gT": ln_gT,
            "ln_bT": ln_bT,
            "bout_bc": np.ascontiguousarray(
                np.broadcast_to(bout[None, c * VS:(c + 1) * VS], (128, VS))).astype(np.float32),
        })
    return in_maps


def run(inputs, debug=False, trace=False):
    key = ("dbg" if debug else "rel")
    if key not in _CACHE:
        _CACHE[key] = build_program(debug=debug)
    nc = _CACHE[key]
    in_maps = _prep_inputs(inputs)
    res = run_bass_kernel_spmd(nc, in_maps, core_ids=list(range(N_CORES)),
                               trace=trace)
    return res


def assemble(res):
    out = np.zeros((B * T, VOCAB), np.float32)
    for c in range(N_CORES):
        lg = res.results[c]["logits"]          # [NVT, 32, 128, VT] bf16
        lg = np.asarray(lg, np.float32).transpose(1, 2, 0, 3).reshape(B * T, VS)
        out[:, c * VS:(c + 1) * VS] = lg
    return out.reshape(B, T, VOCAB)


def kernel(**inputs):
    return assemble(run(inputs))
